# revision 2
# baseline (speedup 1.0000x reference)
"""BilinearAttention Trainium2 kernel — 8-core data-parallel (batch sharded).

Math per batch element b (reference semantics):
  d   = drug @ Wd + bd                     (N=128, HID=512)
  dWb = drug @ (Wd@Wb folded) + bdWb       (N, HID)     [host-folded weights]
  t   = target @ Wt + bt                   (L=1024, HID)
  per head h (HD=64):
    attn = dWb_h @ t_h^T                   (N, L)
    E    = exp(attn + dmask_bias + tmask_bias)          [masked entries -> 0]
    d-side: p_d = E / rowsum(E);  w_d[l] = sum_n p_d * dm[n]/dlen
            ctx_d[h] = sum_l w_d[l] * t_h[l]
    t-side: p_t = E / colsum(E);  w_t[n] = sum_l p_t * tm[l]/tlen
            ctx_t[h] = sum_n w_t[n] * d_h[n]
  out[b] = [ctx_d(512) | ctx_t(512)]

On-chip layout: everything head-transposed (hid on partitions).  targetT via
DMA-transpose (bf16, host-cast).  Softmax normalizers folded into tiny
[128,1]-stationary matmuls; ctx via fused DVE multiply-reduce.
"""

import numpy as np
import ml_dtypes

import concourse.bass as bass
import concourse.bacc as bacc
import concourse.mybir as mybir
from concourse.bass_utils import run_bass_kernel_spmd
from concourse import tile
from concourse.masks import make_identity

NCORES = 8
B = 32
BC = B // NCORES          # 4 batch elements per core
N, L = 128, 1024
KD, KT = 256, 1280        # drug dim, target dim
HID, H, HD = 512, 8, 64
NKC_T = KT // 128         # 10 k-chunks for target proj
NKC_D = KD // 128         # 2 k-chunks for drug proj
NC4 = HID // 128          # 4 hid chunks (2 heads each)
FP32 = mybir.dt.float32
BF16 = mybir.dt.bfloat16
AF = mybir.ActivationFunctionType
ALU = mybir.AluOpType
BF16NP = ml_dtypes.bfloat16


def _body(tc, io):
    nc = tc.nc
    import contextlib
    es = contextlib.ExitStack()

    const = es.enter_context(tc.tile_pool(name="const", bufs=1))

    # ---- weights / constants (loaded once) ----
    wt_t = []
    for kc in range(NKC_T):
        t = const.tile([128, HID], BF16, tag=f"wt{kc}")
        nc.sync.dma_start(out=t[:], in_=io["wt"][kc * 128:(kc + 1) * 128, :])
        wt_t.append(t)
    wd2_t = []
    for kc in range(NKC_D):
        t = const.tile([128, 2 * HID], BF16, tag=f"wd2{kc}")
        nc.sync.dma_start(out=t[:], in_=io["wd2"][kc * 128:(kc + 1) * 128, :])
        wd2_t.append(t)
    ident_f = const.tile([128, 128], FP32, tag="idf")
    make_identity(nc, ident_f[:])
    ones1 = const.tile([1, 128], BF16, tag="ones1")
    nc.vector.memset(ones1[:], 1.0)
    ones64 = const.tile([128, 64], BF16, tag="ones64")
    nc.vector.memset(ones64[:], 1.0)
    btT = const.tile([128, NC4], FP32, tag="btT")
    nc.sync.dma_start(out=btT[:], in_=io["btT"][:])
    b2T = const.tile([128, 8], FP32, tag="b2T")
    nc.sync.dma_start(out=b2T[:], in_=io["b2T"][:])
    dmbT = const.tile([128, BC], FP32, tag="dmbT")
    nc.sync.dma_start(out=dmbT[:], in_=io["dmbT"][:])
    dmwT = const.tile([128, BC], FP32, tag="dmwT")
    nc.sync.dma_start(out=dmwT[:], in_=io["dmwT"][:])
    tmbT_t, tmw64_t, tmb_row_t, dmb_row_t = [], [], [], []
    for b in range(BC):
        t = const.tile([128, 8], FP32, tag=f"tmbT{b}")
        nc.sync.dma_start(out=t[:], in_=io["tmbT"][b])
        tmbT_t.append(t)
        t = const.tile([128, 64], FP32, tag=f"tmw64{b}")
        nc.sync.dma_start(out=t[:], in_=io["tmw64"][b])
        tmw64_t.append(t)
        t = const.tile([1, L], BF16, tag=f"tmbrow{b}")
        nc.sync.dma_start(out=t[:], in_=io["tmb_bf"][b:b + 1, :])
        tmb_row_t.append(t)
        t = const.tile([1, N], BF16, tag=f"dmbrow{b}")
        nc.sync.dma_start(out=t[:], in_=io["dmb_bf"][b:b + 1, :])
        dmb_row_t.append(t)
    ctxT_all = const.tile([128, 128], FP32, tag="ctxall")

    # ---- pools ----
    tgtT_pool = es.enter_context(tc.tile_pool(name="tgtT", bufs=2))
    tT_pool = es.enter_context(tc.tile_pool(name="tT", bufs=2))
    e_pool = es.enter_context(tc.tile_pool(name="E", bufs=12))
    et_pool = es.enter_context(tc.tile_pool(name="Et", bufs=12))
    d2T_pool = es.enter_context(tc.tile_pool(name="d2T", bufs=16))
    small = es.enter_context(tc.tile_pool(name="small", bufs=4))
    junk = es.enter_context(tc.tile_pool(name="junk", bufs=3))
    ps_proj = es.enter_context(tc.tile_pool(name="psproj", bufs=3, space="PSUM"))
    ps_attn = es.enter_context(tc.tile_pool(name="psattn", bufs=5, space="PSUM"))

    for b in range(BC):
        # ---------- targetT via DMA transpose (bf16, from DRAM) ----------
        tgtT = []
        for kc in range(NKC_T):
            t = tgtT_pool.tile([128, L], BF16, tag=f"tgtT{kc}")
            nc.sync.dma_start(
                out=t[:], in_=io["tgt_bf"][b, :, kc * 128:(kc + 1) * 128],
                transpose=True,
            )
            tgtT.append(t)

        if _DEBUG and b == 0:
            for kc in range(NKC_T):
                nc.sync.dma_start(out=io["dbg_tgtT"][kc], in_=tgtT[kc][:])
        # ---------- t projection: tT[c] = (target @ Wt + bt)^T, bf16 ----------
        tT = []
        for c in range(NC4):
            t = tT_pool.tile([128, L], BF16, tag=f"tT{c}")
            tT.append(t)
        for c in range(NC4):
            for lh in range(2):
                ps = ps_proj.tile([128, 512], FP32, tag="psproj")
                for kc in range(NKC_T):
                    nc.tensor.matmul(
                        ps[:],
                        lhsT=wt_t[kc][:, c * 128:(c + 1) * 128],
                        rhs=tgtT[kc][:, lh * 512:(lh + 1) * 512],
                        start=(kc == 0), stop=(kc == NKC_T - 1),
                    )
                nc.scalar.activation(
                    tT[c][:, lh * 512:(lh + 1) * 512], ps[:],
                    AF.Identity, bias=btT[:, c:c + 1],
                )

        if _DEBUG and b == 0:
            for c in range(NC4):
                nc.sync.dma_start(out=io["dbg_tT"][c], in_=tT[c][:])
        # ---------- drug: drugT via DMA transpose; d2T = ([Wd|WdWb]^T drug^T) ----------
        drugT = []
        for kc in range(NKC_D):
            t = d2T_pool.tile([128, N], BF16, tag="drugT", name=f"drugT_{b}_{kc}", bufs=4)
            nc.sync.dma_start(
                out=t[:], in_=io["drug_bf"][b, :, kc * 128:(kc + 1) * 128],
                transpose=True,
            )
            drugT.append(t)
        d2T = [d2T_pool.tile([128, N], BF16, tag="d2T", name=f"d2T_{b}_{i}")
               for i in range(8)]
        for g in range(2):  # pack 4 chunks per psum tile
            ps = ps_attn.tile([128, 512], FP32, tag="psattn")
            for j in range(4):
                ch = g * 4 + j
                for kc in range(NKC_D):
                    nc.tensor.matmul(
                        ps[:, j * 128:(j + 1) * 128],
                        lhsT=wd2_t[kc][:, ch * 128:(ch + 1) * 128],
                        rhs=drugT[kc][:],
                        start=(kc == 0), stop=(kc == NKC_D - 1),
                    )
            for j in range(4):
                ch = g * 4 + j
                nc.scalar.activation(
                    d2T[ch][:], ps[:, j * 128:(j + 1) * 128],
                    AF.Identity, bias=b2T[:, ch:ch + 1],
                )

        if _DEBUG and b == 0:
            for ch in range(8):
                nc.sync.dma_start(out=io["dbg_d2T"][ch], in_=d2T[ch][:])
        # ---------- d-side attention: E = exp(attn + masks), S_d ----------
        E = [e_pool.tile([128, L], BF16, tag="E", name=f"E_{b}_{i}")
             for i in range(H)]
        S_dh = small.tile([128, 16], FP32, tag="Sdh")
        for h in range(H):
            c, ph = h // 2, (h % 2) * 64
            for lh in range(2):
                ps = ps_attn.tile([128, 512], FP32, tag="psattn")
                nc.tensor.matmul(
                    ps[:],
                    lhsT=d2T[4 + c][ph:ph + 64, :],
                    rhs=tT[c][ph:ph + 64, lh * 512:(lh + 1) * 512],
                    start=True, stop=False,
                )
                nc.tensor.matmul(
                    ps[:], lhsT=ones1[:],
                    rhs=tmb_row_t[b][:, lh * 512:(lh + 1) * 512],
                    start=False, stop=True,
                )
                nc.scalar.activation(
                    E[h][:, lh * 512:(lh + 1) * 512], ps[:],
                    AF.Exp, bias=dmbT[:, b:b + 1],
                    accum_out=S_dh[:, lh * 8 + h:lh * 8 + h + 1],
                )
        if _DEBUG and b == 0:
            for h in range(H):
                nc.sync.dma_start(out=io["dbg_E"][h], in_=E[h][:])
            nc.sync.dma_start(out=io["dbg_Sdh"][:], in_=S_dh[:])
        # u = dmask/dlen / S_d
        S_d = small.tile([128, 8], FP32, tag="Sd")
        nc.vector.tensor_tensor(S_d[:], S_dh[:, 0:8], S_dh[:, 8:16], ALU.add)
        nc.vector.tensor_scalar_add(S_d[:], S_d[:], 1e-30)
        recipSd = small.tile([128, 8], FP32, tag="rSd")
        nc.vector.reciprocal(recipSd[:], S_d[:])
        u_f = small.tile([128, 8], FP32, tag="uf")
        nc.vector.tensor_scalar(
            out=u_f[:], in0=recipSd[:], scalar1=dmwT[:, b:b + 1],
            scalar2=None, op0=ALU.mult,
        )
        # u replicated 64-wide per head -> stationary operands
        u_rep = small.tile([128, 8 * 64], BF16, tag="urep")
        for h in range(H):
            nc.vector.tensor_scalar(
                out=u_rep[:, h * 64:(h + 1) * 64], in0=ones64[:],
                scalar1=u_f[:, h:h + 1], scalar2=None, op0=ALU.mult,
            )
        # w_d replicated: ps[(h%2)*64+e, l] = w_d[h, l]; ctx_d by fused reduce
        ctxv = small.tile([128, 8], FP32, tag="ctx")
        acc2 = small.tile([128, 2], FP32, tag="acc2")
        for c in range(NC4):
            for lh in range(2):
                ps = ps_attn.tile([128, 512], FP32, tag="psattn")
                for hp in range(2):
                    h = 2 * c + hp
                    nc.tensor.matmul(
                        ps[hp * 64:(hp + 1) * 64, :],
                        lhsT=u_rep[:, h * 64:(h + 1) * 64],
                        rhs=E[h][:, lh * 512:(lh + 1) * 512],
                        start=True, stop=True,
                    )
                scratch = junk.tile([128, 512], BF16, tag="junk")
                nc.vector.scalar_tensor_tensor(
                    out=scratch[:], in0=ps[:], scalar=1.0,
                    in1=tT[c][:, lh * 512:(lh + 1) * 512],
                    op0=ALU.mult, op1=ALU.mult,
                    accum_out=acc2[:, lh:lh + 1],
                )
            nc.vector.tensor_tensor(
                ctxv[:, c:c + 1], acc2[:, 0:1], acc2[:, 1:2], ALU.add)

        # ---------- t-side attention (transposed layout) ----------
        Et = [et_pool.tile([128, 8 * N], BF16, tag="Et", name=f"Et_{b}_{i}")
              for i in range(8)]
        S_t = small.tile([128, 64], FP32, tag="St")
        for lc in range(8):
            for g4 in range(2):
                ps = ps_attn.tile([128, 512], FP32, tag="psattn")
                for hh in range(4):
                    h = g4 * 4 + hh
                    c, ph = h // 2, (h % 2) * 64
                    nc.tensor.matmul(
                        ps[:, hh * 128:(hh + 1) * 128],
                        lhsT=tT[c][ph:ph + 64, lc * 128:(lc + 1) * 128],
                        rhs=d2T[4 + c][ph:ph + 64, :],
                        start=True, stop=False,
                    )
                    nc.tensor.matmul(
                        ps[:, hh * 128:(hh + 1) * 128],
                        lhsT=ones1[:], rhs=dmb_row_t[b][:],
                        start=False, stop=True,
                    )
                nc.scalar.activation(
                    Et[lc][:, g4 * 512:(g4 + 1) * 512], ps[:],
                    AF.Exp, bias=tmbT_t[b][:, lc:lc + 1],
                )
            nc.vector.tensor_reduce(
                S_t[:, lc * 8:(lc + 1) * 8],
                Et[lc][:].rearrange("p (h n) -> p h n", h=8),
                axis=mybir.AxisListType.X, op=ALU.add,
            )
        # g[l, (lc,h)] = tmask/tlen / S_t
        nc.vector.tensor_scalar_add(S_t[:], S_t[:], 1e-30)
        recipSt = small.tile([128, 64], FP32, tag="rSt")
        nc.vector.reciprocal(recipSt[:], S_t[:])
        g_f = small.tile([128, 64], FP32, tag="gf")
        nc.vector.tensor_tensor(g_f[:], recipSt[:], tmw64_t[b][:], ALU.mult)
        if _DEBUG and b == 0:
            nc.sync.dma_start(out=io["dbg_St"][:], in_=S_t[:])
            nc.sync.dma_start(out=io["dbg_gf"][:], in_=g_f[:])
            nc.sync.dma_start(out=io["dbg_uf"][:], in_=u_f[:])
            for lc in range(8):
                nc.sync.dma_start(out=io["dbg_Et"][lc], in_=Et[lc][:])
        # fold g into Et rows (broadcast per head), in place
        for lc in range(8):
            g_b = g_f[:, lc * 8:(lc + 1) * 8, None].to_broadcast((128, 8, 128))
            nc.vector.tensor_tensor(
                Et[lc][:].rearrange("p (h n) -> p h n", h=8),
                Et[lc][:].rearrange("p (h n) -> p h n", h=8),
                g_b, ALU.mult,
            )
        # w_t replicated via ones-stationary: ps[(h%2)*64+e, n] = w_t[h, n]
        ps_wt = []
        for c in range(NC4):
            ps = ps_attn.tile([128, 512], FP32, tag="psattn")
            ps_wt.append(ps)
        for lc in range(8):
            for h in range(H):
                c, ph = h // 2, (h % 2) * 64
                nc.tensor.matmul(
                    ps_wt[c][ph:ph + 64, 0:128],
                    lhsT=ones64[:],
                    rhs=Et[lc][:, (h // 4) * 512 + (h % 4) * 128:
                               (h // 4) * 512 + (h % 4) * 128 + 128],
                    start=(lc == 0), stop=(lc == 7),
                )
        for c in range(NC4):
            scratch = junk.tile([128, 512], BF16, tag="junk")
            nc.vector.scalar_tensor_tensor(
                out=scratch[:, 0:128], in0=ps_wt[c][:, 0:128], scalar=1.0,
                in1=d2T[c][:], op0=ALU.mult, op1=ALU.mult,
                accum_out=ctxv[:, 4 + c:5 + c],
            )

        if _DEBUG and b == 0:
            nc.sync.dma_start(out=io["dbg_ctx"][:], in_=ctxv[:])
        # ---------- transpose ctx [128, 8] -> [8, 128] and stage ----------
        ps_c = ps_attn.tile([128, 512], FP32, tag="psattn")
        nc.tensor.transpose(ps_c[0:8, 0:128], ctxv[:], ident_f[:])
        nc.scalar.copy(ctxT_all[b * 32:b * 32 + 8, :], ps_c[0:8, 0:128])

    if _DEBUG:
        nc.sync.dma_start(out=io["dbg_ctxT"][:], in_=ctxT_all[:])
    # ---------- output DMA: [32, 128] -> (BC, 1024) ----------
    for b in range(BC):
        nc.sync.dma_start(
            out=io["out"][b].rearrange("(j p) -> j p", j=8),
            in_=ctxT_all[b * 32:b * 32 + 8, :],
        )
    es.close()


_DEBUG = False


def _build():
    nc = bacc.Bacc("TRN2", target_bir_lowering=False, debug=False,
                   num_devices=NCORES)
    io = {}

    def inp(name, shape, dt):
        io[name] = nc.dram_tensor(name, shape, dt, kind="ExternalInput").ap()

    inp("tgt_bf", [BC, L, KT], BF16)
    inp("drug_bf", [BC, N, KD], BF16)
    inp("wt", [KT, HID], BF16)
    inp("wd2", [KD, 2 * HID], BF16)
    inp("btT", [128, NC4], FP32)
    inp("b2T", [128, 8], FP32)
    inp("dmbT", [128, BC], FP32)
    inp("dmwT", [128, BC], FP32)
    inp("tmb_bf", [BC, L], BF16)
    inp("dmb_bf", [BC, N], BF16)
    inp("tmbT", [BC, 128, 8], FP32)
    inp("tmw64", [BC, 128, 64], FP32)
    io["out"] = nc.dram_tensor("out", [BC, 2 * HID], FP32,
                               kind="ExternalOutput").ap()
    if _DEBUG:
        for nm, shape, dt in [
            ("dbg_tgtT", [10, 128, L], BF16),
            ("dbg_tT", [NC4, 128, L], BF16),
            ("dbg_d2T", [8, 128, N], BF16),
            ("dbg_E", [H, 128, L], BF16),
            ("dbg_Et", [8, 128, 8 * N], BF16),
            ("dbg_Sdh", [128, 16], FP32),
            ("dbg_St", [128, 64], FP32),
            ("dbg_uf", [128, 8], FP32),
            ("dbg_gf", [128, 64], FP32),
            ("dbg_ctx", [128, 8], FP32),
            ("dbg_ctxT", [128, 128], FP32),
        ]:
            io[nm] = nc.dram_tensor(nm, shape, dt, kind="ExternalOutput").ap()
    with tile.TileContext(nc) as tc:
        _body(tc, io)
    nc.compile()
    return nc


_NC_CACHE = None
_LAST_RESULTS = None


def _get_nc():
    global _NC_CACHE
    if _NC_CACHE is None:
        _NC_CACHE = _build()
    return _NC_CACHE


def _prep_host(drug_nodes, drug_mask, target_seq, target_mask,
               Wd, bd, Wt, bt, Wb):
    f32 = np.float32
    WdWb = np.einsum("khd,hde->khe", Wd.reshape(KD, H, HD), Wb).reshape(KD, HID)
    bdWb = np.einsum("hd,hde->he", bd.reshape(H, HD), Wb).reshape(HID)
    wd2 = np.ascontiguousarray(
        np.concatenate([Wd, WdWb], axis=1)).astype(BF16NP)
    wt_bf = np.ascontiguousarray(Wt).astype(BF16NP)
    b2 = np.concatenate([bd, bdWb]).astype(f32)
    btT = np.ascontiguousarray(bt.reshape(NC4, 128).T)
    b2T = np.ascontiguousarray(b2.reshape(8, 128).T)
    dlen = np.maximum(drug_mask.sum(-1), 1).astype(f32)
    tlen = np.maximum(target_mask.sum(-1), 1).astype(f32)
    dmb = np.where(drug_mask, 0.0, -1e9).astype(f32)
    tmb = np.where(target_mask, 0.0, -1e9).astype(f32)
    dmw = (drug_mask.astype(f32) / dlen[:, None]).astype(f32)
    tmw = (target_mask.astype(f32) / tlen[:, None]).astype(f32)
    return wd2, wt_bf, btT, b2T, dmb, tmb, dmw, tmw


def kernel(drug_nodes, drug_mask, target_seq, target_mask,
           Wd, bd, Wt, bt, Wb):
    f32 = np.float32
    (wd2, wt_bf, btT, b2T, dmb, tmb, dmw, tmw) = _prep_host(
        np.asarray(drug_nodes, f32), np.asarray(drug_mask),
        np.asarray(target_seq, f32), np.asarray(target_mask),
        np.asarray(Wd, f32), np.asarray(bd, f32), np.asarray(Wt, f32),
        np.asarray(bt, f32), np.asarray(Wb, f32))

    tgt_bf = np.ascontiguousarray(target_seq).astype(BF16NP)
    drug_bf = np.ascontiguousarray(drug_nodes).astype(BF16NP)

    in_maps = []
    for i in range(NCORES):
        s = slice(i * BC, (i + 1) * BC)
        in_maps.append(dict(
            tgt_bf=np.ascontiguousarray(tgt_bf[s]),
            drug_bf=np.ascontiguousarray(drug_bf[s]),
            wt=wt_bf, wd2=wd2, btT=btT, b2T=b2T,
            dmbT=np.ascontiguousarray(dmb[s].T),
            dmwT=np.ascontiguousarray(dmw[s].T),
            tmb_bf=np.ascontiguousarray(tmb[s]).astype(BF16NP),
            dmb_bf=np.ascontiguousarray(dmb[s]).astype(BF16NP),
            tmbT=np.ascontiguousarray(
                tmb[s].reshape(BC, 8, 128).transpose(0, 2, 1)),
            tmw64=np.ascontiguousarray(np.repeat(
                tmw[s].reshape(BC, 8, 128).transpose(0, 2, 1), 8, axis=2)),
        ))

    nc = _get_nc()
    res = run_bass_kernel_spmd(nc, in_maps, list(range(NCORES)))
    global _LAST_RESULTS
    _LAST_RESULTS = res
    out = np.concatenate([res.results[i]["out"] for i in range(NCORES)],
                         axis=0)
    return np.ascontiguousarray(out.astype(np.float32))

